# revision 25
# baseline (speedup 1.0000x reference)
"""EuclideanDeconf kernel for 8x TRN2 NeuronCores.

Computes out[b, c] = (2/D) * x @ W.T - ||x||^2/D - ||W||^2/D
for x [16384, 1024] f32, W [2048, 1024] f32 -> out [16384, 2048] f32.

Sharding: data-parallel over the batch dim. Each of the 8 cores gets 2048
rows of x and the full W. The host does layout-only work (transpose /
cast / shard / concat); all FLOPs (matmul, row/col norms, combine) run
on device.

I/O dtypes (ring total 16 MiB/core ~= 46us, under the PE's ~57us fp8
GEMM+extras floor):
  xT8  e4m3 x16-prescaled, b-256-blocked [8, p, k, 256] (matmul lhsT,
       straight from DMA to LDWEIGHTS -- no on-device casts)
  xT16 f16, chunk-blocked [4, p, k, 512] (x^2 path; ~1e-5 rel err)
  wT   e4m3 x16-prescaled, p-major [p, k, c]
  y    f16, host-upcast to f32 (~5e-4 max rel)
All host layouts are p-major with the transferred block contiguous per
partition, so every DMA is a 2D pattern (128 descriptors, 2-16KB each):
descriptor generation on the Sync engine stays ~0.7us per transfer
(3D patterns cost 3-4.5us each and serialized the old startup).

Measured vs the fp32 reference: max rel err ~1.2e-3, norm ~2.6e-4
(gate 2e-2).

Ring order: x16-ch0 (x^2 chain has the longest latency: gpsimd square
-> DVE tree -> PE dot -> ACT copy feeds pass1's bias), then W k0k1 +
xT8 blk0 (first matmuls), then the rest of W/x interleaved.

Engine schedule (per core):
  PE:   8 warmup, then b-tiles 0..15 back-to-back: 16 DR matmuls + x2
        dot each; after b7: 16 DR w2-reduce + 4 replicate matmuls.
  ACT:  per tile: x2c copy + pass1 (t = s*psum - x2[b], f16); one W^2
        square slice after pass1 on odd tiles b1..b7 (keeps ACT cadence
        ~= PE cadence; pass1 is what frees PSUM for the PE).
  DVE:  x2 k-trees; W^2 squares k0..k3 interleaved with chunk-0 trees;
        pass2 (y = t - w2[c], all-f16) inline from b8, deferred b0..b7
        (epool=20 t-tiles of runway) until w2rep exists.
  GPSIMD: x^2 squares from f16.
"""

import numpy as np
import ml_dtypes

# Problem constants (hardcoded; kernel.py must be self-contained).
B, D, C = 16384, 1024, 2048
NCORES = 8
BSH = B // NCORES  # 2048 rows of x per core
P = 128            # partitions
KT = D // P        # 8 contraction tiles
BCH = 512          # x16 chunk columns
NCH = BSH // BCH   # 4 chunks
BLK = 256          # xT8 block columns (2 b-tiles)
NBLK = BSH // BLK  # 8 blocks
JT = BCH // P      # 4 b-tiles per chunk
NJ = BSH // P      # 16 b-tiles

_CACHE = {}


def _build_nc():
    import concourse.tile as tile
    import concourse.mybir as mybir
    import concourse.bass as bass
    from concourse import bacc

    f32 = mybir.dt.float32
    f16 = mybir.dt.float16
    bf16 = mybir.dt.bfloat16
    fp8 = mybir.dt.float8e4
    PSUM = bass.MemorySpace.PSUM
    Identity = mybir.ActivationFunctionType.Identity
    Copy = mybir.ActivationFunctionType.Copy
    Square = mybir.ActivationFunctionType.Square
    MULT = mybir.AluOpType.mult
    ADD = mybir.AluOpType.add
    DR = mybir.MatmulPerfMode.DoubleRow

    # x and W both host-prescaled by 16 (keeps e4m3 out of subnormals);
    # the epilogue scales fold the 1/256 back out.
    cross_scale = 2.0 / D / 256.0
    w2_scale = 1.0 / D / 256.0

    nc = bacc.Bacc(
        "TRN2",
        target_bir_lowering=False,
        debug=False,
        enable_asserts=False,
    )
    xT8 = nc.dram_tensor("xT8", [NBLK * P, KT * BLK], fp8,
                         kind="ExternalInput").ap()
    xT16 = nc.dram_tensor("xT16", [NJ * P, KT * P], f16,
                          kind="ExternalInput").ap()
    wT = nc.dram_tensor("wT", [P, KT * C], fp8, kind="ExternalInput").ap()
    y = nc.dram_tensor("y", [BSH, C], f16, kind="ExternalOutput").ap()

    xT8r = xT8.rearrange("(s p) (k b) -> s p k b", p=P, k=KT)
    xT16r = xT16.rearrange("(s p) (k b) -> s p k b", p=P, k=KT)
    wTr = wT.rearrange("p (k c) -> p k c", k=KT)

    with tile.TileContext(nc) as tc:
        with (
            tc.tile_pool(name="consts", bufs=1) as cpool,
            tc.tile_pool(name="wpool", bufs=1) as wpool,
            tc.tile_pool(name="xpool", bufs=1) as xpool,
            tc.tile_pool(name="x16pool", bufs=3) as x16pool,
            tc.tile_pool(name="xsqpool", bufs=3) as xsqpool,
            tc.tile_pool(name="epool", bufs=24) as epool,
            tc.tile_pool(name="ypool", bufs=6) as ypool,
            tc.tile_pool(name="spool", bufs=8) as spool,
            tc.tile_pool(name="pmain", bufs=3, space=PSUM) as pmain,
            tc.tile_pool(name="psmall", bufs=1, space=PSUM) as psmall,
        ):
            negones_f = cpool.tile([P, 1], f32)
            nc.gpsimd.memset(negones_f[:], -1.0)
            negones_b = cpool.tile([P, 1], bf16)
            nc.gpsimd.memset(negones_b[:], -1.0)
            nego2 = cpool.tile([P, 2, P], fp8)
            nc.gpsimd.memset(nego2[:], -1.0)
            ones1_b = cpool.tile([1, P], bf16)
            nc.gpsimd.memset(ones1_b[:], 1.0)
            warm = cpool.tile([1, 1], f32)
            # touch ACT early so its function-table DMA (~2.7us) is off the
            # critical path by the time the first pass1 runs
            nc.scalar.activation(warm[:], negones_f[0:1, 0:1], Identity,
                                 bias=0.0, scale=1.0)

            # ---- PE warmup: dummy matmuls so HAM un-throttles by the time
            # real work arrives (fits inside the chunk-0 DMA wait) ----
            warm_b = cpool.tile([P, 512], bf16)
            nc.gpsimd.memset(warm_b[:], 0.0)
            # tiny 1-col matmuls wake the PE clock at ~1/4 the PE-busy cost
            # of full 512-col dummies (PE duty is HAM-limited)
            warm_ps = psmall.tile([P, 512], f32, tag="w2ps", bufs=1)
            for _ in range(8):
                nc.tensor.matmul(warm_ps[:, 0:1], warm_b[:, 0:P],
                                 warm_b[:, 0:1], start=True, stop=True)

            # xbf blocked [p, blk, k, 256] so each block DMA lands in a
            # contiguous 2KB-per-partition stripe
            xbf = xpool.tile([P, NBLK, KT, BLK], fp8)
            wbf = wpool.tile([P, KT, C], fp8)
            xf16s = {}

            def dma_x16(j):
                xf = x16pool.tile([P, KT, P], f16, tag="xf",
                                  name=f"xf{j}", bufs=6)
                nc.sync.dma_start(xf[:], xT16r[j])
                xf16s[j] = xf

            def dma_xbf(blk):
                nc.sync.dma_start(xbf[:, blk, :, :], xT8r[blk])

            def dma_w(g):
                nc.sync.dma_start(wbf[:, 2 * g:2 * g + 2, :],
                                  wTr[:, 2 * g:2 * g + 2, :])

            # ring order: first-matmul operands lead (W k0k1 + x8 blk0),
            # x2-chain tiles ride between W groups (the chain has ~4us of
            # slack while W paces b0); everything 2D-contiguous
            dma_w(0)
            dma_xbf(0)
            dma_x16(0)
            dma_w(1)
            dma_x16(1)
            dma_xbf(1)
            dma_w(2)
            dma_w(3)
            dma_x16(2)
            dma_x16(3)
            for j in range(4, 8):
                dma_x16(j)
            dma_xbf(2)
            dma_xbf(3)
            for j in range(8, 12):
                dma_x16(j)
            for blk in range(4, NBLK):
                dma_xbf(blk)
            for j in range(12, NJ):
                dma_x16(j)

            # ---- per-b-tile x^2 partials: GPSIMD squares + DVE k-trees;
            # t1 in bf16 so the x2-dot LDWEIGHTS runs at 1 cyc/row ----
            t1s = {}

            def prep_sq(ch):
                for jj in range(JT):
                    j = ch * JT + jj
                    xf = xf16s.pop(j)
                    xsq = xsqpool.tile([P, KT, P], f32, tag="xsq",
                                       name=f"xsq{j}")
                    nc.gpsimd.tensor_tensor(xsq[:], xf[:], xf[:], op=MULT)
                    t4 = xsqpool.tile([P, 4, P], f32, tag="t4",
                                      name=f"t4_{j}")
                    nc.vector.tensor_tensor(t4[:], xsq[:, 0:4, :],
                                            xsq[:, 4:8, :], op=ADD)
                    t2 = xsqpool.tile([P, 2, P], f32, tag="t2",
                                      name=f"t2_{j}")
                    nc.vector.tensor_tensor(t2[:], t4[:, 0:2, :],
                                            t4[:, 2:4, :], op=ADD)
                    t1 = xsqpool.tile([P, P], bf16, tag="t1", bufs=8,
                                      name=f"t1_{j}")
                    nc.vector.tensor_tensor(t1[:], t2[:, 0, :], t2[:, 1, :],
                                            op=ADD)
                    t1s[j] = t1
                    yield j

            # ---- W^2 squares: e4m3 in/out, k0..k3 on DVE (interleaved with
            # chunk-0 trees), k4..k7 on ACT (after odd-tile pass1s) ----
            wsq = wpool.tile([P, KT, C], fp8)

            def wsq_dve(k):
                nc.vector.tensor_tensor(wsq[:, k, :], wbf[:, k, :],
                                        wbf[:, k, :], op=MULT)

            def wsq_gps(k):
                nc.gpsimd.tensor_tensor(wsq[:, k, :], wbf[:, k, :],
                                        wbf[:, k, :], op=MULT)

            def wsq_act(k):
                nc.scalar.activation(wsq[:, k, :], wbf[:, k, :], Square)

            for _ in prep_sq(0):
                pass
            wsq_gps(2)              # gpsimd: between chunk-0 and -1 squares
            wsq_gps(3)
            for i, _ in enumerate(prep_sq(1)):
                if i >= 2:
                    wsq_dve(i - 2)  # DVE: k0,k1 after trees b6,b7

            # ---- per-b-tile pieces ----
            y_bufs = {}
            t_bufs = {}

            def btile_matmuls(jg):
                """The 16 accumulating DR matmuls for one 128-row b-tile."""
                ps0 = pmain.tile([P, 1024], f32, tag="ps", name=f"ps{jg}a")
                ps1 = pmain.tile([P, 1024], f32, tag="ps", name=f"ps{jg}b")
                pss = (ps0, ps0, ps1, ps1)
                blk, sub = jg // 2, jg % 2
                for k2 in range(KT // 2):
                    lhsT = xbf[:, blk, 2 * k2:2 * k2 + 2,
                               sub * P:(sub + 1) * P]
                    for cj in range(4):
                        nc.tensor.matmul(
                            pss[cj][:, (cj % 2) * 512:(cj % 2) * 512 + 512],
                            lhsT,
                            wbf[:, 2 * k2:2 * k2 + 2, cj * 512:(cj + 1) * 512],
                            start=(k2 == 0),
                            stop=(k2 == KT // 2 - 1),
                            perf_mode=DR,
                        )
                y_bufs[jg] = (ps0, ps1)

            def x2_col(jg):
                """x2 column (-sum(x^2)/D) for one b-tile: PE dot + ACT copy."""
                t1 = t1s.pop(jg)
                x2ps = psmall.tile([P, 1], f32, tag="x2ps", bufs=1,
                                   name=f"x2ps{jg}")
                nc.tensor.matmul(x2ps[:], t1[:], negones_b[:],
                                 start=True, stop=True)
                x2c = spool.tile([P, 1], f32, tag="x2c", name=f"x2c{jg}")
                nc.scalar.activation(x2c[:], x2ps[:], Copy, bias=0.0,
                                     scale=1.0 / D)
                return x2c

            def btile_pass1(jg, x2c):
                """ACT: t = cross_scale*psum - x2[b]  (f16 out, drains PSUM)."""
                ps0, ps1 = y_bufs.pop(jg)
                ts = []
                for h, psh in enumerate((ps0, ps1)):
                    t = epool.tile([P, 1024], f16, tag="t", name=f"t{jg}_{h}")
                    nc.scalar.activation(t[:], psh[:], Identity,
                                         bias=x2c[:], scale=cross_scale)
                    ts.append(t)
                t_bufs[jg] = ts

            def btile_pass2(jg, split=False):
                """DVE: y = t - w2rep (all f16) + store.

                split: store in 512-col quarters right behind each pass2 so
                the tail (last b-tile) overlaps epilogue and DMA maximally.
                """
                ts = t_bufs.pop(jg)
                y_t = ypool.tile([P, C], f16, tag="y_t", name=f"y_t{jg}")
                for h in range(2):
                    for q in range(2 if split else 1):
                        lo = h * 1024 + q * 512
                        hi = h * 1024 + (q + 1) * 512 if split else (h + 1) * 1024
                        ysl = y_t[:, lo:hi]
                        nc.vector.tensor_add(
                            ysl, ts[h][:, lo - h * 1024:hi - h * 1024],
                            w2rep[:, lo:hi]
                        )
                        if split:
                            nc.sync.dma_start(
                                y[jg * P:(jg + 1) * P, lo:hi], ysl,
                            )
                if not split:
                    nc.sync.dma_start(y[jg * P:(jg + 1) * P, :], y_t[:])

            def w2_finish():
                """DR reduce of wsq + broadcast: w2rep [P, C] f16."""
                w2row = wpool.tile([1, C], bf16)
                for cj in range(4):
                    w2ps = psmall.tile([P, 512], f32, tag="w2ps", bufs=1,
                                       name=f"w2ps{cj}")
                    for k2 in range(KT // 2):
                        nc.tensor.matmul(
                            w2ps[:],
                            nego2[:],
                            wsq[:, 2 * k2:2 * k2 + 2,
                                cj * 512:(cj + 1) * 512],
                            start=(k2 == 0),
                            stop=(k2 == KT // 2 - 1),
                            perf_mode=DR,
                        )
                    # w2row = -sum(W^2)/D (every PSUM row holds the sum)
                    nc.scalar.activation(w2row[:, cj * 512:(cj + 1) * 512],
                                         w2ps[0:1, :], Copy, bias=0.0,
                                         scale=w2_scale)
                rep = wpool.tile([P, C], f16)
                for cj in range(4):
                    w2rp = psmall.tile([P, 512], f32, tag="w2ps", bufs=1,
                                       name=f"w2rp{cj}")
                    nc.tensor.matmul(w2rp[:], ones1_b[:],
                                     w2row[:, cj * 512:(cj + 1) * 512],
                                     start=True, stop=True)
                    nc.scalar.activation(rep[:, cj * 512:(cj + 1) * 512],
                                         w2rp[:], Copy, bias=0.0, scale=1.0)
                return rep

            # ---- main per-b-tile loop ----
            w2rep = None
            for j in range(NJ):
                btile_matmuls(j)
                x2c = x2_col(j)
                btile_pass1(j, x2c)
                if j in (1, 3, 5, 7):
                    wsq_act(4 + j // 2)     # k4..k7 on ACT, odd tiles
                if j == 7:
                    for _ in prep_sq(2):
                        pass
                if j == 8:
                    w2rep = w2_finish()
                    for jj in range(9):     # deferred chunk-0/1/2 stores
                        btile_pass2(jj)
                    for _ in prep_sq(3):
                        pass
                if j >= 9:
                    btile_pass2(j, split=(j == NJ - 1))

    nc.compile()
    return nc


def _get_nc():
    if "nc" not in _CACHE:
        _CACHE["nc"] = _build_nc()
    return _CACHE["nc"]


def _prep_inputs(x, W):
    x = np.ascontiguousarray(x, dtype=np.float32)
    W = np.ascontiguousarray(W, dtype=np.float32)
    # W -> [p, k, c] p-major, x16 prescale, e4m3
    wp = W.reshape(C, KT, P).transpose(2, 1, 0)  # [p, k, c]
    wT = (np.ascontiguousarray(wp) * np.float32(16.0)).astype(
        ml_dtypes.float8_e4m3).reshape(P, KT * C)
    in_maps = []
    for i in range(NCORES):
        xs = x[i * BSH:(i + 1) * BSH, :]             # [BSH, D]
        # xT8: b-256-blocked [blk, p, k, b], x16 prescale
        x8 = xs.reshape(NBLK, BLK, KT, P).transpose(0, 3, 2, 1)
        xT8_i = (np.ascontiguousarray(x8) * np.float32(16.0)).astype(
            ml_dtypes.float8_e4m3).reshape(NBLK * P, KT * BLK)
        # xT16: b-tile-blocked [j, p, k, b]
        x16 = xs.reshape(NJ, P, KT, P).transpose(0, 3, 2, 1)
        xT16_i = np.ascontiguousarray(x16).astype(np.float16).reshape(
            NJ * P, KT * P)
        in_maps.append({"xT8": xT8_i, "xT16": xT16_i, "wT": wT})
    return in_maps


def run(x, W, trace=False, **trace_kwargs):
    """Run on the 8 cores; returns (out [B, C] f32, BassKernelResults)."""
    from concourse import bass_utils

    nc = _get_nc()
    in_maps = _prep_inputs(x, W)
    res = bass_utils.run_bass_kernel_spmd(
        nc, in_maps, core_ids=list(range(NCORES)), trace=trace, **trace_kwargs
    )
    out = np.concatenate(
        [r["y"].astype(np.float32) for r in res.results], axis=0
    )
    return out, res


def kernel(x, W, task_id=None, **_unused):
    out, _ = run(np.asarray(x), np.asarray(W), trace=False)
    return out


# revision 28
# speedup vs baseline: 1.0245x; 1.0245x over previous
"""EuclideanDeconf kernel for 8x TRN2 NeuronCores.

Computes out[b, c] = (2/D) * x @ W.T - ||x||^2/D - ||W||^2/D
for x [16384, 1024] f32, W [2048, 1024] f32 -> out [16384, 2048] f32.

Sharding: data-parallel over the batch dim. Each of the 8 cores gets 2048
rows of x and the full W. The host does layout-only work (transpose /
cast / shard / concat); all FLOPs (matmul, row/col norms, combine) run
on device.

I/O dtypes (ring total 16 MiB/core ~= 46us, under the PE's ~57us fp8
GEMM+extras floor):
  xT8  e4m3 x16-prescaled, b-256-blocked [8, p, k, 256] (matmul lhsT,
       straight from DMA to LDWEIGHTS -- no on-device casts)
  xT16 f16, chunk-blocked [4, p, k, 512] (x^2 path; ~1e-5 rel err)
  wT   e4m3 x16-prescaled, p-major [p, k, c]
  y    f16, host-upcast to f32 (~5e-4 max rel)
All host layouts are p-major with the transferred block contiguous per
partition, so every DMA is a 2D pattern (128 descriptors, 2-16KB each):
descriptor generation on the Sync engine stays ~0.7us per transfer
(3D patterns cost 3-4.5us each and serialized the old startup).

Measured vs the fp32 reference: max rel err ~1.2e-3, norm ~2.6e-4
(gate 2e-2).

Ring order: x16-ch0 (x^2 chain has the longest latency: gpsimd square
-> DVE tree -> PE dot -> ACT copy feeds pass1's bias), then W k0k1 +
xT8 blk0 (first matmuls), then the rest of W/x interleaved.

Engine schedule (per core):
  PE:   8 warmup, then b-tiles 0..15 back-to-back: 16 DR matmuls + x2
        dot each; after b7: 16 DR w2-reduce + 4 replicate matmuls.
  ACT:  per tile: x2c copy + pass1 (t = s*psum - x2[b], f16); one W^2
        square slice after pass1 on odd tiles b1..b7 (keeps ACT cadence
        ~= PE cadence; pass1 is what frees PSUM for the PE).
  DVE:  x2 k-trees; W^2 squares k0..k3 interleaved with chunk-0 trees;
        pass2 (y = t - w2[c], all-f16) inline from b8, deferred b0..b7
        (epool=20 t-tiles of runway) until w2rep exists.
  GPSIMD: x^2 squares from f16.
"""

import numpy as np
import ml_dtypes

# Problem constants (hardcoded; kernel.py must be self-contained).
B, D, C = 16384, 1024, 2048
NCORES = 8
BSH = B // NCORES  # 2048 rows of x per core
P = 128            # partitions
KT = D // P        # 8 contraction tiles
BCH = 512          # x16 chunk columns
NCH = BSH // BCH   # 4 chunks
BLK = 256          # xT8 block columns (2 b-tiles)
NBLK = BSH // BLK  # 8 blocks
JT = BCH // P      # 4 b-tiles per chunk
NJ = BSH // P      # 16 b-tiles

_CACHE = {}


def _build_nc():
    import concourse.tile as tile
    import concourse.mybir as mybir
    import concourse.bass as bass
    from concourse import bacc

    f32 = mybir.dt.float32
    f16 = mybir.dt.float16
    bf16 = mybir.dt.bfloat16
    fp8 = mybir.dt.float8e4
    PSUM = bass.MemorySpace.PSUM
    Identity = mybir.ActivationFunctionType.Identity
    Copy = mybir.ActivationFunctionType.Copy
    Square = mybir.ActivationFunctionType.Square
    MULT = mybir.AluOpType.mult
    ADD = mybir.AluOpType.add
    DR = mybir.MatmulPerfMode.DoubleRow

    # x and W both host-prescaled by 16 (keeps e4m3 out of subnormals);
    # the epilogue scales fold the 1/256 back out.
    cross_scale = 2.0 / D / 256.0
    w2_scale = 1.0 / D / 256.0

    nc = bacc.Bacc(
        "TRN2",
        target_bir_lowering=False,
        debug=False,
        enable_asserts=False,
    )
    xT8 = nc.dram_tensor("xT8", [NBLK * P, KT * BLK], fp8,
                         kind="ExternalInput").ap()
    xT16 = nc.dram_tensor("xT16", [NJ * P, KT * P], f16,
                          kind="ExternalInput").ap()
    wT = nc.dram_tensor("wT", [P, KT * C], fp8, kind="ExternalInput").ap()
    y = nc.dram_tensor("y", [BSH, C], f16, kind="ExternalOutput").ap()

    xT8r = xT8.rearrange("(s p) (k b) -> s p k b", p=P, k=KT)
    xT16r = xT16.rearrange("(s p) (k b) -> s p k b", p=P, k=KT)
    wTr = wT.rearrange("p (k c) -> p k c", k=KT)

    with tile.TileContext(nc) as tc:
        with (
            tc.tile_pool(name="consts", bufs=1) as cpool,
            tc.tile_pool(name="wpool", bufs=1) as wpool,
            tc.tile_pool(name="xpool", bufs=1) as xpool,
            tc.tile_pool(name="x16pool", bufs=3) as x16pool,
            tc.tile_pool(name="xsqpool", bufs=3) as xsqpool,
            tc.tile_pool(name="epool", bufs=28) as epool,
            tc.tile_pool(name="ypool", bufs=6) as ypool,
            tc.tile_pool(name="spool", bufs=8) as spool,
            tc.tile_pool(name="pmain", bufs=3, space=PSUM) as pmain,
            tc.tile_pool(name="psmall", bufs=1, space=PSUM) as psmall,
        ):
            negones_f = cpool.tile([P, 1], f32)
            nc.gpsimd.memset(negones_f[:], -1.0)
            negones_b = cpool.tile([P, 1], bf16)
            nc.gpsimd.memset(negones_b[:], -1.0)
            nego2 = cpool.tile([P, 2, P], fp8)
            nc.gpsimd.memset(nego2[:], -1.0)
            ones1_b = cpool.tile([1, P], bf16)
            nc.gpsimd.memset(ones1_b[:], 1.0)
            warm = cpool.tile([1, 1], f32)
            # touch ACT early so its function-table DMA (~2.7us) is off the
            # critical path by the time the first pass1 runs
            nc.scalar.activation(warm[:], negones_f[0:1, 0:1], Identity,
                                 bias=0.0, scale=1.0)

            # ---- PE warmup: dummy matmuls so HAM un-throttles by the time
            # real work arrives (fits inside the chunk-0 DMA wait) ----
            warm_b = cpool.tile([P, 512], bf16)
            nc.gpsimd.memset(warm_b[:], 0.0)
            # tiny 1-col matmuls wake the PE clock at ~1/4 the PE-busy cost
            # of full 512-col dummies (PE duty is HAM-limited)
            warm_ps = psmall.tile([P, 512], f32, tag="w2ps", bufs=1)
            for _ in range(8):
                nc.tensor.matmul(warm_ps[:, 0:1], warm_b[:, 0:P],
                                 warm_b[:, 0:1], start=True, stop=True)

            # xbf blocked [p, blk, k, 256] so each block DMA lands in a
            # contiguous 2KB-per-partition stripe
            xbf = xpool.tile([P, NBLK, KT, BLK], fp8)
            wbf = wpool.tile([P, KT, C], fp8)
            xf16s = {}

            def dma_x16(j):
                xf = x16pool.tile([P, KT, P], f16, tag="xf",
                                  name=f"xf{j}", bufs=6)
                nc.sync.dma_start(xf[:], xT16r[j])
                xf16s[j] = xf

            def dma_xbf(blk):
                nc.sync.dma_start(xbf[:, blk, :, :], xT8r[blk])

            def dma_w(g):
                nc.sync.dma_start(wbf[:, 2 * g:2 * g + 2, :],
                                  wTr[:, 2 * g:2 * g + 2, :])

            # ring order: first-matmul operands lead (W k0k1 + x8 blk0),
            # x2-chain tiles ride between W groups (the chain has ~4us of
            # slack while W paces b0); everything 2D-contiguous
            dma_w(0)
            dma_xbf(0)
            dma_x16(0)
            dma_w(1)
            dma_x16(1)
            dma_xbf(1)
            dma_w(2)
            dma_w(3)
            dma_x16(2)
            dma_x16(3)
            for j in range(4, 8):
                dma_x16(j)
            dma_xbf(2)
            dma_xbf(3)
            for j in range(8, 12):
                dma_x16(j)
            for blk in range(4, NBLK):
                dma_xbf(blk)
            for j in range(12, NJ):
                dma_x16(j)

            # ---- per-b-tile x^2 partials: GPSIMD squares + DVE k-trees;
            # t1 in bf16 so the x2-dot LDWEIGHTS runs at 1 cyc/row ----
            t1s = {}

            def prep_sq(ch):
                for jj in range(JT):
                    j = ch * JT + jj
                    xf = xf16s.pop(j)
                    xsq = xsqpool.tile([P, KT, P], f32, tag="xsq",
                                       name=f"xsq{j}")
                    nc.gpsimd.tensor_tensor(xsq[:], xf[:], xf[:], op=MULT)
                    t4 = xsqpool.tile([P, 4, P], f32, tag="t4",
                                      name=f"t4_{j}")
                    nc.vector.tensor_tensor(t4[:], xsq[:, 0:4, :],
                                            xsq[:, 4:8, :], op=ADD)
                    t2 = xsqpool.tile([P, 2, P], f32, tag="t2",
                                      name=f"t2_{j}")
                    nc.vector.tensor_tensor(t2[:], t4[:, 0:2, :],
                                            t4[:, 2:4, :], op=ADD)
                    t1 = xsqpool.tile([P, P], bf16, tag="t1", bufs=8,
                                      name=f"t1_{j}")
                    nc.vector.tensor_tensor(t1[:], t2[:, 0, :], t2[:, 1, :],
                                            op=ADD)
                    t1s[j] = t1
                    yield j

            # ---- W^2 squares: e4m3 in/out, k0..k3 on DVE (interleaved with
            # chunk-0 trees), k4..k7 on ACT (after odd-tile pass1s) ----
            wsq = wpool.tile([P, KT, C], fp8)

            def wsq_act(k):
                nc.scalar.activation(wsq[:, k, :], wbf[:, k, :], Square)

            # k0,k1 in ACT's idle window before the first pass1 (fp8 squares
            # are ~2us/slice on ACT vs ~4-6us on DVE/GPSIMD)
            wsq_act(0)
            wsq_act(1)
            for _ in prep_sq(0):
                pass
            for _ in prep_sq(1):
                pass

            # ---- per-b-tile pieces ----
            y_bufs = {}
            t_bufs = {}

            def btile_matmuls(jg):
                """The 16 accumulating DR matmuls for one 128-row b-tile."""
                ps0 = pmain.tile([P, 1024], f32, tag="ps", name=f"ps{jg}a")
                ps1 = pmain.tile([P, 1024], f32, tag="ps", name=f"ps{jg}b")
                pss = (ps0, ps0, ps1, ps1)
                blk, sub = jg // 2, jg % 2
                for k2 in range(KT // 2):
                    lhsT = xbf[:, blk, 2 * k2:2 * k2 + 2,
                               sub * P:(sub + 1) * P]
                    for cj in range(4):
                        nc.tensor.matmul(
                            pss[cj][:, (cj % 2) * 512:(cj % 2) * 512 + 512],
                            lhsT,
                            wbf[:, 2 * k2:2 * k2 + 2, cj * 512:(cj + 1) * 512],
                            start=(k2 == 0),
                            stop=(k2 == KT // 2 - 1),
                            perf_mode=DR,
                        )
                y_bufs[jg] = (ps0, ps1)

            def x2_col(jg):
                """x2 column (-sum(x^2)/D) for one b-tile: PE dot + ACT copy."""
                t1 = t1s.pop(jg)
                x2ps = psmall.tile([P, 1], f32, tag="x2ps", bufs=1,
                                   name=f"x2ps{jg}")
                nc.tensor.matmul(x2ps[:], t1[:], negones_b[:],
                                 start=True, stop=True)
                x2c = spool.tile([P, 1], f32, tag="x2c", name=f"x2c{jg}")
                nc.scalar.activation(x2c[:], x2ps[:], Copy, bias=0.0,
                                     scale=1.0 / D)
                return x2c

            def btile_pass1(jg, x2c):
                """ACT: t = cross_scale*psum - x2[b]  (f16 out, drains PSUM)."""
                ps0, ps1 = y_bufs.pop(jg)
                ts = []
                for h, psh in enumerate((ps0, ps1)):
                    t = epool.tile([P, 1024], f16, tag="t", name=f"t{jg}_{h}")
                    nc.scalar.activation(t[:], psh[:], Identity,
                                         bias=x2c[:], scale=cross_scale)
                    ts.append(t)
                t_bufs[jg] = ts

            def btile_pass2(jg, split=False):
                """DVE: y = t - w2rep (all f16) + store.

                split: store in 512-col quarters right behind each pass2 so
                the tail (last b-tile) overlaps epilogue and DMA maximally.
                """
                ts = t_bufs.pop(jg)
                y_t = ypool.tile([P, C], f16, tag="y_t", name=f"y_t{jg}")
                for h in range(2):
                    for q in range(2 if split else 1):
                        lo = h * 1024 + q * 512
                        hi = h * 1024 + (q + 1) * 512 if split else (h + 1) * 1024
                        ysl = y_t[:, lo:hi]
                        nc.vector.tensor_add(
                            ysl, ts[h][:, lo - h * 1024:hi - h * 1024],
                            w2rep[:, lo:hi]
                        )
                        if split:
                            nc.sync.dma_start(
                                y[jg * P:(jg + 1) * P, lo:hi], ysl,
                            )
                if not split:
                    nc.sync.dma_start(y[jg * P:(jg + 1) * P, :], y_t[:])

            def w2_finish():
                """DR reduce of wsq + broadcast: w2rep [P, C] f16."""
                w2row = wpool.tile([1, C], bf16)
                for cj in range(4):
                    w2ps = psmall.tile([P, 512], f32, tag="w2ps", bufs=1,
                                       name=f"w2ps{cj}")
                    for k2 in range(KT // 2):
                        nc.tensor.matmul(
                            w2ps[:],
                            nego2[:],
                            wsq[:, 2 * k2:2 * k2 + 2,
                                cj * 512:(cj + 1) * 512],
                            start=(k2 == 0),
                            stop=(k2 == KT // 2 - 1),
                            perf_mode=DR,
                        )
                    # w2row = -sum(W^2)/D (every PSUM row holds the sum)
                    nc.scalar.activation(w2row[:, cj * 512:(cj + 1) * 512],
                                         w2ps[0:1, :], Copy, bias=0.0,
                                         scale=w2_scale)
                rep = wpool.tile([P, C], f16)
                for cj in range(4):
                    w2rp = psmall.tile([P, 512], f32, tag="w2ps", bufs=1,
                                       name=f"w2rp{cj}")
                    nc.tensor.matmul(w2rp[:], ones1_b[:],
                                     w2row[:, cj * 512:(cj + 1) * 512],
                                     start=True, stop=True)
                    nc.scalar.activation(rep[:, cj * 512:(cj + 1) * 512],
                                         w2rp[:], Copy, bias=0.0, scale=1.0)
                return rep

            # ---- main per-b-tile loop ----
            w2rep = None
            for j in range(NJ):
                btile_matmuls(j)
                x2c = x2_col(j)
                btile_pass1(j, x2c)
                if j in (1, 3, 5, 7, 8, 9):
                    # k2..k7 spread so ACT's per-tile load stays ~= the
                    # PE's b-tile cadence (pass1 is what frees PSUM)
                    wsq_act(2 + (j // 2 if j < 8 else j - 5))
                if j == 7:
                    for _ in prep_sq(2):
                        pass
                if j == 11:
                    w2rep = w2_finish()
                    for _ in prep_sq(3):
                        pass
                    for jj in range(12):    # deferred stores b0..b11
                        btile_pass2(jj)
                if j >= 12:
                    btile_pass2(j, split=(j == NJ - 1))

    nc.compile()
    return nc


def _get_nc():
    if "nc" not in _CACHE:
        _CACHE["nc"] = _build_nc()
    return _CACHE["nc"]


def _prep_inputs(x, W):
    x = np.ascontiguousarray(x, dtype=np.float32)
    W = np.ascontiguousarray(W, dtype=np.float32)
    # W -> [p, k, c] p-major, x16 prescale, e4m3
    wp = W.reshape(C, KT, P).transpose(2, 1, 0)  # [p, k, c]
    wT = (np.ascontiguousarray(wp) * np.float32(16.0)).astype(
        ml_dtypes.float8_e4m3).reshape(P, KT * C)
    in_maps = []
    for i in range(NCORES):
        xs = x[i * BSH:(i + 1) * BSH, :]             # [BSH, D]
        # xT8: b-256-blocked [blk, p, k, b], x16 prescale
        x8 = xs.reshape(NBLK, BLK, KT, P).transpose(0, 3, 2, 1)
        xT8_i = (np.ascontiguousarray(x8) * np.float32(16.0)).astype(
            ml_dtypes.float8_e4m3).reshape(NBLK * P, KT * BLK)
        # xT16: b-tile-blocked [j, p, k, b]
        x16 = xs.reshape(NJ, P, KT, P).transpose(0, 3, 2, 1)
        xT16_i = np.ascontiguousarray(x16).astype(np.float16).reshape(
            NJ * P, KT * P)
        in_maps.append({"xT8": xT8_i, "xT16": xT16_i, "wT": wT})
    return in_maps


def run(x, W, trace=False, **trace_kwargs):
    """Run on the 8 cores; returns (out [B, C] f32, BassKernelResults)."""
    from concourse import bass_utils

    nc = _get_nc()
    in_maps = _prep_inputs(x, W)
    res = bass_utils.run_bass_kernel_spmd(
        nc, in_maps, core_ids=list(range(NCORES)), trace=trace, **trace_kwargs
    )
    out = np.concatenate(
        [r["y"].astype(np.float32) for r in res.results], axis=0
    )
    return out, res


def kernel(x, W, task_id=None, **_unused):
    out, _ = run(np.asarray(x), np.asarray(W), trace=False)
    return out


# revision 31
# speedup vs baseline: 1.0314x; 1.0068x over previous
"""EuclideanDeconf kernel for 8x TRN2 NeuronCores.

Computes out[b, c] = (2/D) * x @ W.T - ||x||^2/D - ||W||^2/D
for x [16384, 1024] f32, W [2048, 1024] f32 -> out [16384, 2048] f32.

Sharding: data-parallel over the batch dim. Each of the 8 cores gets 2048
rows of x and the full W. The host does layout-only work (transpose /
cast / shard / concat); all FLOPs (matmul, row/col norms, combine) run
on device.

I/O dtypes (ring total 16 MiB/core ~= 46us, under the PE's ~57us fp8
GEMM+extras floor):
  xT8  e4m3 x16-prescaled, b-256-blocked [8, p, k, 256] (matmul lhsT,
       straight from DMA to LDWEIGHTS -- no on-device casts)
  xT16 f16, chunk-blocked [4, p, k, 512] (x^2 path; ~1e-5 rel err)
  wT   e4m3 x16-prescaled, p-major [p, k, c]
  y    f16, host-upcast to f32 (~5e-4 max rel)
All host layouts are p-major with the transferred block contiguous per
partition, so every DMA is a 2D pattern (128 descriptors, 2-16KB each):
descriptor generation on the Sync engine stays ~0.7us per transfer
(3D patterns cost 3-4.5us each and serialized the old startup).

Measured vs the fp32 reference: max rel err ~1.2e-3, norm ~2.6e-4
(gate 2e-2).

Ring order: x16-ch0 (x^2 chain has the longest latency: gpsimd square
-> DVE tree -> PE dot -> ACT copy feeds pass1's bias), then W k0k1 +
xT8 blk0 (first matmuls), then the rest of W/x interleaved.

Engine schedule (per core):
  PE:   8 warmup, then b-tiles 0..15 back-to-back: 16 DR matmuls + x2
        dot each; after b7: 16 DR w2-reduce + 4 replicate matmuls.
  ACT:  per tile: x2c copy + pass1 (t = s*psum - x2[b], f16); one W^2
        square slice after pass1 on odd tiles b1..b7 (keeps ACT cadence
        ~= PE cadence; pass1 is what frees PSUM for the PE).
  DVE:  x2 k-trees; W^2 squares k0..k3 interleaved with chunk-0 trees;
        pass2 (y = t - w2[c], all-f16) inline from b8, deferred b0..b7
        (epool=20 t-tiles of runway) until w2rep exists.
  GPSIMD: x^2 squares from f16.
"""

import numpy as np
import ml_dtypes

# Problem constants (hardcoded; kernel.py must be self-contained).
B, D, C = 16384, 1024, 2048
NCORES = 8
BSH = B // NCORES  # 2048 rows of x per core
P = 128            # partitions
KT = D // P        # 8 contraction tiles
BCH = 512          # x16 chunk columns
NCH = BSH // BCH   # 4 chunks
BLK = 256          # xT8 block columns (2 b-tiles)
NBLK = BSH // BLK  # 8 blocks
JT = BCH // P      # 4 b-tiles per chunk
NJ = BSH // P      # 16 b-tiles

_CACHE = {}


def _build_nc():
    import concourse.tile as tile
    import concourse.mybir as mybir
    import concourse.bass as bass
    from concourse import bacc

    f32 = mybir.dt.float32
    f16 = mybir.dt.float16
    bf16 = mybir.dt.bfloat16
    fp8 = mybir.dt.float8e4
    PSUM = bass.MemorySpace.PSUM
    Identity = mybir.ActivationFunctionType.Identity
    Copy = mybir.ActivationFunctionType.Copy
    Square = mybir.ActivationFunctionType.Square
    MULT = mybir.AluOpType.mult
    ADD = mybir.AluOpType.add
    DR = mybir.MatmulPerfMode.DoubleRow

    # x and W both host-prescaled by 16 (keeps e4m3 out of subnormals);
    # the epilogue scales fold the 1/256 back out.
    cross_scale = 2.0 / D / 256.0
    w2_scale = 1.0 / D / 256.0

    nc = bacc.Bacc(
        "TRN2",
        target_bir_lowering=False,
        debug=False,
        enable_asserts=False,
    )
    xT8 = nc.dram_tensor("xT8", [NBLK * P, KT * BLK], fp8,
                         kind="ExternalInput").ap()
    xT16 = nc.dram_tensor("xT16", [NJ * P, KT * P], f16,
                          kind="ExternalInput").ap()
    wT = nc.dram_tensor("wT", [P, KT * C], fp8, kind="ExternalInput").ap()
    y = nc.dram_tensor("y", [BSH, C], f16, kind="ExternalOutput").ap()

    xT8r = xT8.rearrange("(s p) (k b) -> s p k b", p=P, k=KT)
    xT16r = xT16.rearrange("(s p) (k b) -> s p k b", p=P, k=KT)
    wTr = wT.rearrange("p (k c) -> p k c", k=KT)

    with tile.TileContext(nc) as tc:
        with (
            tc.tile_pool(name="consts", bufs=1) as cpool,
            tc.tile_pool(name="wpool", bufs=1) as wpool,
            tc.tile_pool(name="xpool", bufs=1) as xpool,
            tc.tile_pool(name="x16pool", bufs=3) as x16pool,
            tc.tile_pool(name="xsqpool", bufs=3) as xsqpool,
            tc.tile_pool(name="epool", bufs=28) as epool,
            tc.tile_pool(name="ypool", bufs=6) as ypool,
            tc.tile_pool(name="spool", bufs=8) as spool,
            tc.tile_pool(name="pmain", bufs=3, space=PSUM) as pmain,
            tc.tile_pool(name="psmall", bufs=1, space=PSUM) as psmall,
        ):
            negones_f = cpool.tile([P, 1], f32)
            nc.gpsimd.memset(negones_f[:], -1.0)
            negones_b = cpool.tile([P, 1], bf16)
            nc.gpsimd.memset(negones_b[:], -1.0)
            nego2 = cpool.tile([P, 2, P], fp8)
            nc.gpsimd.memset(nego2[:], -1.0)
            ones1_b = cpool.tile([1, P], bf16)
            nc.gpsimd.memset(ones1_b[:], 1.0)
            warm = cpool.tile([1, 1], f32)
            # touch ACT early so its function-table DMA (~2.7us) is off the
            # critical path by the time the first pass1 runs
            nc.scalar.activation(warm[:], negones_f[0:1, 0:1], Identity,
                                 bias=0.0, scale=1.0)

            # ---- PE warmup: dummy matmuls so HAM un-throttles by the time
            # real work arrives (fits inside the chunk-0 DMA wait) ----
            warm_b = cpool.tile([P, 512], bf16)
            nc.gpsimd.memset(warm_b[:], 0.0)
            # tiny 1-col matmuls wake the PE clock at ~1/4 the PE-busy cost
            # of full 512-col dummies (PE duty is HAM-limited)
            warm_ps = psmall.tile([P, 512], f32, tag="w2ps", bufs=1)
            for _ in range(8):
                nc.tensor.matmul(warm_ps[:, 0:1], warm_b[:, 0:P],
                                 warm_b[:, 0:1], start=True, stop=True)

            # xbf blocked [p, blk, k, 256] so each block DMA lands in a
            # contiguous 2KB-per-partition stripe
            xbf = xpool.tile([P, NBLK, KT, BLK], fp8)
            wbf = wpool.tile([P, KT, C], fp8)
            xf16s = {}

            def dma_x16(j):
                xf = x16pool.tile([P, KT, P], f16, tag="xf",
                                  name=f"xf{j}", bufs=6)
                nc.sync.dma_start(xf[:], xT16r[j])
                xf16s[j] = xf

            def dma_xbf(blk):
                nc.sync.dma_start(xbf[:, blk, :, :], xT8r[blk])

            def dma_w(g):
                nc.sync.dma_start(wbf[:, 2 * g:2 * g + 2, :],
                                  wTr[:, 2 * g:2 * g + 2, :])

            # ring order: first-matmul operands lead (W k0k1 + x8 blk0),
            # x2-chain tiles ride between W groups (the chain has ~4us of
            # slack while W paces b0); everything 2D-contiguous
            dma_w(0)
            dma_xbf(0)
            dma_x16(0)
            dma_w(1)
            dma_x16(1)
            dma_xbf(1)
            dma_w(2)
            dma_w(3)
            dma_x16(2)
            dma_x16(3)
            for j in range(4, 8):
                dma_x16(j)
            dma_xbf(2)
            dma_xbf(3)
            for j in range(8, 12):
                dma_x16(j)
            for blk in range(4, NBLK):
                dma_xbf(blk)
            for j in range(12, NJ):
                dma_x16(j)

            # ---- per-b-tile x^2 partials: GPSIMD squares + DVE k-trees;
            # t1 in bf16 so the x2-dot LDWEIGHTS runs at 1 cyc/row ----
            t1s = {}

            def prep_sq(ch):
                for jj in range(JT):
                    j = ch * JT + jj
                    xf = xf16s.pop(j)
                    xsq = xsqpool.tile([P, KT, P], f32, tag="xsq",
                                       name=f"xsq{j}")
                    nc.gpsimd.tensor_tensor(xsq[:], xf[:], xf[:], op=MULT)
                    t4 = xsqpool.tile([P, 4, P], f32, tag="t4",
                                      name=f"t4_{j}")
                    nc.vector.tensor_tensor(t4[:], xsq[:, 0:4, :],
                                            xsq[:, 4:8, :], op=ADD)
                    t2 = xsqpool.tile([P, 2, P], f32, tag="t2",
                                      name=f"t2_{j}")
                    nc.vector.tensor_tensor(t2[:], t4[:, 0:2, :],
                                            t4[:, 2:4, :], op=ADD)
                    t1 = xsqpool.tile([P, P], bf16, tag="t1", bufs=8,
                                      name=f"t1_{j}")
                    nc.vector.tensor_tensor(t1[:], t2[:, 0, :], t2[:, 1, :],
                                            op=ADD)
                    t1s[j] = t1
                    yield j

            # ---- W^2 squares: e4m3 in/out, k0..k3 on DVE (interleaved with
            # chunk-0 trees), k4..k7 on ACT (after odd-tile pass1s) ----
            wsq = wpool.tile([P, KT, C], fp8)

            def wsq_act(k):
                nc.scalar.activation(wsq[:, k, :], wbf[:, k, :], Square)

            def wsq_dve(k):
                nc.vector.tensor_tensor(wsq[:, k, :], wbf[:, k, :],
                                        wbf[:, k, :], op=MULT)

            # k2,k3 in ACT's idle window before the first pass1 (fp8 squares
            # are ~2us/slice on ACT vs ~4us on DVE); k0,k1 on DVE placed
            # after the chunk-0 trees so the early x2 dots never wait
            wsq_act(2)
            wsq_act(3)
            for _ in prep_sq(0):
                pass
            wsq_dve(0)
            for i, _ in enumerate(prep_sq(1)):
                if i == 0:
                    wsq_dve(1)

            # ---- per-b-tile pieces ----
            y_bufs = {}
            t_bufs = {}

            def btile_matmuls(jg):
                """The 16 accumulating DR matmuls for one 128-row b-tile."""
                ps0 = pmain.tile([P, 1024], f32, tag="ps", name=f"ps{jg}a")
                ps1 = pmain.tile([P, 1024], f32, tag="ps", name=f"ps{jg}b")
                pss = (ps0, ps0, ps1, ps1)
                blk, sub = jg // 2, jg % 2
                for k2 in range(KT // 2):
                    lhsT = xbf[:, blk, 2 * k2:2 * k2 + 2,
                               sub * P:(sub + 1) * P]
                    for cj in range(4):
                        nc.tensor.matmul(
                            pss[cj][:, (cj % 2) * 512:(cj % 2) * 512 + 512],
                            lhsT,
                            wbf[:, 2 * k2:2 * k2 + 2, cj * 512:(cj + 1) * 512],
                            start=(k2 == 0),
                            stop=(k2 == KT // 2 - 1),
                            perf_mode=DR,
                        )
                y_bufs[jg] = (ps0, ps1)

            def x2_col(jg):
                """x2 column (-sum(x^2)/D) for one b-tile: PE dot + ACT copy."""
                t1 = t1s.pop(jg)
                x2ps = psmall.tile([P, 1], f32, tag="x2ps", bufs=1,
                                   name=f"x2ps{jg}")
                nc.tensor.matmul(x2ps[:], t1[:], negones_b[:],
                                 start=True, stop=True)
                x2c = spool.tile([P, 1], f32, tag="x2c", name=f"x2c{jg}")
                nc.scalar.activation(x2c[:], x2ps[:], Copy, bias=0.0,
                                     scale=1.0 / D)
                return x2c

            def btile_pass1(jg, x2c):
                """ACT: t = cross_scale*psum - x2[b]  (f16 out, drains PSUM)."""
                ps0, ps1 = y_bufs.pop(jg)
                ts = []
                for h, psh in enumerate((ps0, ps1)):
                    t = epool.tile([P, 1024], f16, tag="t", name=f"t{jg}_{h}")
                    nc.scalar.activation(t[:], psh[:], Identity,
                                         bias=x2c[:], scale=cross_scale)
                    ts.append(t)
                t_bufs[jg] = ts

            def btile_pass2(jg, split=False):
                """DVE: y = t - w2rep (all f16) + store.

                split: store in 512-col quarters right behind each pass2 so
                the tail (last b-tile) overlaps epilogue and DMA maximally.
                """
                ts = t_bufs.pop(jg)
                y_t = ypool.tile([P, C], f16, tag="y_t", name=f"y_t{jg}")
                for h in range(2):
                    for q in range(2 if split else 1):
                        lo = h * 1024 + q * 512
                        hi = h * 1024 + (q + 1) * 512 if split else (h + 1) * 1024
                        ysl = y_t[:, lo:hi]
                        nc.vector.tensor_add(
                            ysl, ts[h][:, lo - h * 1024:hi - h * 1024],
                            w2rep[:, lo:hi]
                        )
                        if split:
                            nc.sync.dma_start(
                                y[jg * P:(jg + 1) * P, lo:hi], ysl,
                            )
                if not split:
                    nc.sync.dma_start(y[jg * P:(jg + 1) * P, :], y_t[:])

            def w2_finish():
                """DR reduce of wsq + broadcast: w2rep [P, C] f16."""
                w2row = wpool.tile([1, C], bf16)
                for cj in range(4):
                    w2ps = psmall.tile([P, 512], f32, tag="w2ps", bufs=1,
                                       name=f"w2ps{cj}")
                    for k2 in range(KT // 2):
                        nc.tensor.matmul(
                            w2ps[:],
                            nego2[:],
                            wsq[:, 2 * k2:2 * k2 + 2,
                                cj * 512:(cj + 1) * 512],
                            start=(k2 == 0),
                            stop=(k2 == KT // 2 - 1),
                            perf_mode=DR,
                        )
                    # w2row = -sum(W^2)/D (every PSUM row holds the sum)
                    nc.scalar.activation(w2row[:, cj * 512:(cj + 1) * 512],
                                         w2ps[0:1, :], Copy, bias=0.0,
                                         scale=w2_scale)
                rep = wpool.tile([P, C], f16)
                for cj in range(4):
                    w2rp = psmall.tile([P, 512], f32, tag="w2ps", bufs=1,
                                       name=f"w2rp{cj}")
                    nc.tensor.matmul(w2rp[:], ones1_b[:],
                                     w2row[:, cj * 512:(cj + 1) * 512],
                                     start=True, stop=True)
                    nc.scalar.activation(rep[:, cj * 512:(cj + 1) * 512],
                                         w2rp[:], Copy, bias=0.0, scale=1.0)
                return rep

            # ---- main per-b-tile loop ----
            w2rep = None
            for j in range(NJ):
                btile_matmuls(j)
                x2c = x2_col(j)
                btile_pass1(j, x2c)
                if j in (1, 3, 5, 7):
                    # k4..k7 on odd tiles so ACT's per-tile load stays ~=
                    # the PE's b-tile cadence (pass1 is what frees PSUM)
                    wsq_act(4 + (j - 1) // 2)
                if j == 7:
                    for _ in prep_sq(2):
                        pass
                if j == 8:
                    w2rep = w2_finish()
                    for _ in prep_sq(3):
                        pass
                    for jj in range(9):     # deferred stores b0..b8
                        btile_pass2(jj)
                if j >= 9:
                    btile_pass2(j, split=(j == NJ - 1))

    nc.compile()
    return nc


def _get_nc():
    if "nc" not in _CACHE:
        _CACHE["nc"] = _build_nc()
    return _CACHE["nc"]


def _prep_inputs(x, W):
    x = np.ascontiguousarray(x, dtype=np.float32)
    W = np.ascontiguousarray(W, dtype=np.float32)
    # W -> [p, k, c] p-major, x16 prescale, e4m3
    wp = W.reshape(C, KT, P).transpose(2, 1, 0)  # [p, k, c]
    wT = (np.ascontiguousarray(wp) * np.float32(16.0)).astype(
        ml_dtypes.float8_e4m3).reshape(P, KT * C)
    in_maps = []
    for i in range(NCORES):
        xs = x[i * BSH:(i + 1) * BSH, :]             # [BSH, D]
        # xT8: b-256-blocked [blk, p, k, b], x16 prescale
        x8 = xs.reshape(NBLK, BLK, KT, P).transpose(0, 3, 2, 1)
        xT8_i = (np.ascontiguousarray(x8) * np.float32(16.0)).astype(
            ml_dtypes.float8_e4m3).reshape(NBLK * P, KT * BLK)
        # xT16: b-tile-blocked [j, p, k, b]
        x16 = xs.reshape(NJ, P, KT, P).transpose(0, 3, 2, 1)
        xT16_i = np.ascontiguousarray(x16).astype(np.float16).reshape(
            NJ * P, KT * P)
        in_maps.append({"xT8": xT8_i, "xT16": xT16_i, "wT": wT})
    return in_maps


def run(x, W, trace=False, **trace_kwargs):
    """Run on the 8 cores; returns (out [B, C] f32, BassKernelResults)."""
    from concourse import bass_utils

    nc = _get_nc()
    in_maps = _prep_inputs(x, W)
    res = bass_utils.run_bass_kernel_spmd(
        nc, in_maps, core_ids=list(range(NCORES)), trace=trace, **trace_kwargs
    )
    out = np.concatenate(
        [r["y"].astype(np.float32) for r in res.results], axis=0
    )
    return out, res


def kernel(x, W, task_id=None, **_unused):
    out, _ = run(np.asarray(x), np.asarray(W), trace=False)
    return out
